# revision 2
# baseline (speedup 1.0000x reference)
"""Trainium2 Bass kernel for nn_FACoef.

Math: reference computes s_i(b) = sum of all entries of x_b^(i+2) for
i in 0..3, then out[b] = sum_ij coef[i,j] * s_i^(j+1) / n^(i+j+2).

Key identity: 1^T x^(i+2) 1 = ((x^T)^(i+1) 1) . (x 1), so with the
column-sum chain c_k = (x^T)^k 1 (per-sample TensorE matvecs, x as the
fp32 stationary operand) and row-sums r1 = x 1 (VectorE free-axis
reduce), s_i = c_{i+1} . r1. That is O(B N^2) instead of the
reference's O(B N^3) matrix powers.

Sharding: pure data parallel - batch dim of x split across 8 cores,
256 samples per core. The tiny coef enters only through a host-scaled
[4,4] table W and per-row scale sc (n^beta balancing, so no fp32
overflow when raising s_i to the 4th power on device).
"""

import numpy as np

B, N = 2048, 128
ROWS, COLS = 4, 4
N_CORES = 8
BPC = B // N_CORES  # samples per core
S = 32              # samples per sbuf tile
T = BPC // S        # tiles per core

_cache = {}


def _patch_tail_drain():
    """walrus CoreV3 setupSyncWait rejects instructions carrying several
    semaphore waits; TileContext's kernel-tail drain collects one wait per
    unobserved logical proc. Split them one wait per drain instruction."""
    import concourse.tile as tile
    from concourse import mybir
    from concourse.vector_clock import ScopedClock

    if getattr(tile.TileContext, "_drain_split_patched", False):
        return

    def _drain_and_barrier(self, tick_clock, wait_clock):
        nc = self.nc
        drain_inst = nc.sync.drain()
        wait_clock.add_sem_waits(
            drain_inst.ins, ScopedClock({None: tick_clock.global_clock})
        )
        si = drain_inst.ins.sync_info
        waits = list(si.on_wait) if si is not None and si.on_wait else []
        if len(waits) > 1:
            drain_inst.ins.sync_info = mybir.SyncInfo(
                on_wait=[waits[0]], on_update=list(si.on_update or [])
            )
            for w in waits[1:]:
                extra = nc.sync.drain()
                extra.ins.sync_info = mybir.SyncInfo(on_wait=[w], on_update=[])

        nc.all_engine_barrier()
        assert self.sems is not None
        popped = nc._tile_sem_poison_stack.pop()
        assert popped is self._sem_poison
        nc.clear_and_free_semaphores(list(self.sems.allocated().values()))
        nc.all_engine_barrier()

    tile.TileContext._drain_and_barrier = _drain_and_barrier
    tile.TileContext._drain_split_patched = True


def _split_multi_waits(nc):
    """walrus accepts at most one sync wait per instruction (two for
    EventSemaphore). Hoist extra waits onto same-engine NOPs inserted
    immediately before the carrying instruction."""
    from concourse import mybir

    n_split = 0
    for bb in nc.main_func.blocks:
        new = []
        for inst in bb.instructions:
            si = inst.sync_info
            waits = list(si.on_wait) if si is not None and si.on_wait else []
            cap = 2 if isinstance(inst, mybir.InstEventSemaphore) else 1
            if len(waits) > cap:
                for k, w in enumerate(waits[:-cap]):
                    nop = mybir.InstNoOp(name=f"{inst.name}-wsplit{k}", ins=[], outs=[])
                    nop.engine = inst.engine
                    nop.sync_info = mybir.SyncInfo(on_wait=[w], on_update=[])
                    nc.register_instruction(nop)
                    new.append(nop)
                    n_split += 1
                inst.sync_info = mybir.SyncInfo(
                    on_wait=waits[-cap:], on_update=list(si.on_update or [])
                )
            new.append(inst)
        bb.instructions[:] = new
    return n_split


def _consolidate_pe_incs(nc):
    """Every TensorE matmul carries a +1 semaphore inc (~26 ns issue tail
    each). Consumers only wait at era boundaries, so batch the increments:
    strip per-mm incs and emit one accumulated inc at each waited value."""
    from concourse import mybir

    waited = {}
    for bb in nc.main_func.blocks:
        for ins in bb.instructions:
            si = ins.sync_info
            if si is None:
                continue
            for w in si.on_wait or []:
                waited.setdefault(w.ant_name, set()).add(w.wait_value)

    for bb in nc.main_func.blocks:
        pe_insts = [
            ins
            for ins in bb.instructions
            if isinstance(ins, mybir.InstMatmult)
            and ins.sync_info is not None
            and ins.sync_info.on_update
        ]
        by_sem = {}
        for ins in pe_insts:
            for u in ins.sync_info.on_update:
                if u.update_mode == "sem-inc":
                    by_sem.setdefault(u.ant_name, []).append((ins, u))
        import bisect

        for sem, pairs in by_sem.items():
            wvals = waited.get(sem, set())
            cum = 0
            kept_cums = []
            for idx, (ins, u) in enumerate(pairs):
                cum += u.update_value
                keep = cum in wvals or idx == len(pairs) - 1
                si = ins.sync_info
                others = [
                    x
                    for x in si.on_update
                    if not (x.ant_name == sem and x.update_mode == "sem-inc")
                ]
                if keep:
                    others.append(u)  # original +1 inc (hw requires value 1)
                    kept_cums.append(cum)
                ins.sync_info = mybir.SyncInfo(
                    on_wait=list(si.on_wait or []), on_update=others
                )
            # remap every wait on this sem from raw counts to kept-inc counts
            for bb2 in nc.main_func.blocks:
                for ins in bb2.instructions:
                    si = ins.sync_info
                    if si is None or not si.on_wait:
                        continue
                    changed = False
                    new_waits = []
                    for w in si.on_wait:
                        if w.ant_name == sem and w.wait_mode == "sem-ge-imm":
                            n = bisect.bisect_left(kept_cums, w.wait_value) + 1
                            assert n <= len(kept_cums), (sem, w.wait_value)
                            new_waits.append(
                                mybir.SyncWait(
                                    sync_type="semaphore",
                                    id=w.id,
                                    ant_name=sem,
                                    wait_mode="sem-ge-imm",
                                    wait_value=n,
                                )
                            )
                            changed = True
                        else:
                            new_waits.append(w)
                    if changed:
                        ins.sync_info = mybir.SyncInfo(
                            on_wait=new_waits, on_update=list(si.on_update or [])
                        )


def _build_nc(reps=1, use_bf16=True, era1_bf16=True):
    import concourse.bass as bass
    import concourse.tile as tile
    from concourse import mybir

    _patch_tail_drain()
    f32 = mybir.dt.float32
    bf16 = mybir.dt.bfloat16
    AX = mybir.AxisListType
    OP = mybir.AluOpType

    nc = bass.Bass()
    x_in = nc.declare_dram_parameter("x", [BPC, N, N], f32, isOutput=False)
    w_in = nc.declare_dram_parameter("w", [ROWS, COLS], f32, isOutput=False)
    sc_in = nc.declare_dram_parameter("sc", [ROWS, 1], f32, isOutput=False)
    y_out = nc.declare_dram_parameter("y", [1, BPC], f32, isOutput=True)

    x_ibj = x_in.rearrange("b i j -> i b j")  # [N, BPC, N] view

    with tile.TileContext(nc) as tc:
        with (
            tc.tile_pool(name="xp", bufs=(4 if use_bf16 else T)) as xp,
            tc.tile_pool(name="xbp", bufs=T) as xbp,
            tc.tile_pool(name="cp", bufs=4) as cp,
            tc.tile_pool(name="rp", bufs=4) as rp,
            tc.tile_pool(name="constp", bufs=1) as constp,
            tc.tile_pool(name="smallp", bufs=1) as smallp,
            tc.tile_pool(name="psp", bufs=3, space="PSUM") as psp,
            tc.tile_pool(name="psdp", bufs=2, space="PSUM") as psdp,
            tc.tile_pool(name="psfp", bufs=1, space="PSUM") as psfp,
        ):
            ones = constp.tile([N, 1], f32)
            nc.vector.memset(ones[:], 1.0)
            onesb = constp.tile([N, 1], bf16)
            nc.vector.memset(onesb[:], 1.0)
            ones4 = constp.tile([ROWS, 1], f32)
            nc.vector.memset(ones4[:], 1.0)
            wt = constp.tile([ROWS, COLS], f32)
            nc.sync.dma_start(wt[:], w_in[:])
            sct = constp.tile([ROWS, 1], f32)
            nc.sync.dma_start(sct[:], sc_in[:])

            xf = {}

            def dma_tile(t):
                xf[t] = xp.tile([N, S, N], f32, name="xt", tag="xt")
                nc.sync.dma_start(xf[t][:, :, :], x_ibj[:, t * S : (t + 1) * S, :])

            if not use_bf16:
                for t in range(T):
                    dma_tile(t)

            for _rep in range(reps):
                if use_bf16:
                    for t in (0, 1):
                        dma_tile(t)
                sg = smallp.tile([ROWS, BPC], f32, name="sg", tag="sg")

                # tiles processed in pairs with eras interleaved (a,b,a,b..):
                # tile b's era-k matmuls hide tile a's psum->sbuf copy
                # latency, so TensorE never stalls on an era boundary.
                # bf16 mode: chain matvecs use bf16 stationaries (fast
                # weight load hides behind matmul issue); c-vectors for the
                # dot products and r1 stay fp32. Output err ~1.5e-3 with
                # era1_bf16 (~1.4e-4 without) vs the ~2e-2 gate.
                for tp in range(T // 2):
                    pair = (2 * tp, 2 * tp + 1)
                    if use_bf16 and tp + 1 < T // 2:
                        for t in (2 * tp + 2, 2 * tp + 3):
                            dma_tile(t)
                    cts, ctbs, r1s, xtb = {}, {}, {}, {}
                    for t in pair:
                        cts[t] = cp.tile([N, 4 * S], f32, name="ct", tag="ct")
                        r1s[t] = rp.tile([N, S], f32, name="r1", tag="r1")
                        if use_bf16:
                            ctbs[t] = cp.tile(
                                [N, 3 * S], bf16, name="ctb", tag="ctb"
                            )
                    # era 1 (fp32 unless era1_bf16) + r1 + bf16 cast of x
                    for t in pair:
                        xt = xf[t]
                        ct = cts[t]
                        if use_bf16 and era1_bf16:
                            xtb[t] = xbp.tile(
                                [N, S, N], bf16, name="xtb", tag="xtb"
                            )
                            nc.scalar.copy(xtb[t][:, :, :], xt[:, :, :])
                        ps = psp.tile([N, S], f32, name="ps", tag="ps")
                        for b in range(S):
                            if use_bf16 and era1_bf16:
                                nc.tensor.matmul(
                                    ps[:, b : b + 1],
                                    xtb[t][:, b, :],
                                    onesb[:, 0:1],
                                )
                            else:
                                nc.tensor.matmul(
                                    ps[:, b : b + 1], xt[:, b, :], ones[:, 0:1]
                                )
                        nc.vector.tensor_copy(ct[:, 0:S], ps[:])
                        if use_bf16:
                            nc.scalar.copy(ctbs[t][:, 0:S], ps[:])
                            if not era1_bf16:
                                xtb[t] = xbp.tile(
                                    [N, S, N], bf16, name="xtb", tag="xtb"
                                )
                                nc.scalar.copy(xtb[t][:, :, :], xt[:, :, :])
                        nc.vector.tensor_reduce(
                            r1s[t][:], xt[:, :, :], axis=AX.X, op=OP.add
                        )
                    # eras 2-4
                    for k in range(1, 4):
                        for t in pair:
                            ct = cts[t]
                            ps = psp.tile([N, S], f32, name="ps", tag="ps")
                            if use_bf16:
                                src, vec = xtb[t], ctbs[t]
                            else:
                                src, vec = xf[t], cts[t]
                            for b in range(S):
                                nc.tensor.matmul(
                                    ps[:, b : b + 1],
                                    src[:, b, :],
                                    vec[:, (k - 1) * S + b : (k - 1) * S + b + 1],
                                )
                            nc.vector.tensor_copy(ct[:, k * S : (k + 1) * S], ps[:])
                            if use_bf16 and k < 3:
                                nc.scalar.copy(
                                    ctbs[t][:, k * S : (k + 1) * S], ps[:]
                                )
                    # dots: s_i(b) = r1(b) . c_{i+1}(b); the four fp32
                    # c-columns as stationary put s_0..s_3 on partitions 0-3
                    for t in pair:
                        ct, r1 = cts[t], r1s[t]
                        psd = psdp.tile([ROWS, S], f32, name="psd", tag="psd")
                        for b in range(S):
                            nc.tensor.matmul(
                                psd[:, b : b + 1],
                                ct[:, b :: S],
                                r1[:, b : b + 1],
                            )
                        nc.vector.tensor_copy(sg[:, t * S : (t + 1) * S], psd[:])

                # poly epilogue on [4, BPC]: out = sum_ij W[i,j] sig_i^(j+1)
                sig = smallp.tile([ROWS, BPC], f32, name="sig", tag="sig")
                nc.vector.tensor_scalar_mul(sig[:], sg[:], sct[:, 0:1])
                sig2 = smallp.tile([ROWS, BPC], f32, name="sig2", tag="sig2")
                nc.vector.tensor_mul(sig2[:], sig[:], sig[:])
                sig3 = smallp.tile([ROWS, BPC], f32, name="sig3", tag="sig3")
                nc.vector.tensor_mul(sig3[:], sig2[:], sig[:])
                sig4 = smallp.tile([ROWS, BPC], f32, name="sig4", tag="sig4")
                nc.vector.tensor_mul(sig4[:], sig2[:], sig2[:])
                acc1 = smallp.tile([ROWS, BPC], f32, name="acc1", tag="acc1")
                nc.vector.tensor_scalar_mul(acc1[:], sig[:], wt[:, 0:1])
                acc2 = smallp.tile([ROWS, BPC], f32, name="acc2", tag="acc2")
                nc.vector.scalar_tensor_tensor(
                    acc2[:], sig2[:], wt[:, 1:2], acc1[:], op0=OP.mult, op1=OP.add
                )
                acc3 = smallp.tile([ROWS, BPC], f32, name="acc3", tag="acc3")
                nc.vector.scalar_tensor_tensor(
                    acc3[:], sig3[:], wt[:, 2:3], acc2[:], op0=OP.mult, op1=OP.add
                )
                acc4 = smallp.tile([ROWS, BPC], f32, name="acc4", tag="acc4")
                nc.vector.scalar_tensor_tensor(
                    acc4[:], sig4[:], wt[:, 3:4], acc3[:], op0=OP.mult, op1=OP.add
                )
                psf = psfp.tile([1, BPC], f32, name="psf", tag="psf")
                nc.tensor.matmul(psf[0:1, :], ones4[:, 0:1], acc4[:])
                outt = smallp.tile([1, BPC], f32, name="outt", tag="outt")
                nc.vector.tensor_copy(outt[:], psf[:])
                nc.sync.dma_start(y_out[:], outt[:])

    _consolidate_pe_incs(nc)
    _split_multi_waits(nc)
    return nc


def _host_tables(coef):
    n = np.float64(N * N)
    ii = np.arange(ROWS, dtype=np.float64)[:, None]
    jj = np.arange(COLS, dtype=np.float64)[None, :]
    beta = (ii + 3.0) / 4.0
    w = (coef.astype(np.float64) * n ** (beta * (jj + 1.0) - (ii + jj + 2.0))).astype(
        np.float32
    )
    sc = (n ** (-beta[:, 0:1])).astype(np.float32)
    return w, sc


def _in_maps(x, coef):
    w, sc = _host_tables(np.asarray(coef))
    x = np.ascontiguousarray(np.asarray(x, dtype=np.float32))
    return [
        {"x": x[c * BPC : (c + 1) * BPC], "w": w, "sc": sc} for c in range(N_CORES)
    ]


def kernel(x, coef):
    from concourse.bass_utils import run_bass_kernel_spmd

    if "nc" not in _cache:
        _cache["nc"] = _build_nc()
    nc = _cache["nc"]

    in_maps = _in_maps(x, coef)
    res = run_bass_kernel_spmd(nc, in_maps, list(range(N_CORES)))
    y = np.concatenate(
        [np.asarray(res.results[c]["y"]).reshape(-1) for c in range(N_CORES)]
    )
    return y.astype(np.float32)



# revision 3
# speedup vs baseline: 5.9611x; 5.9611x over previous
"""Trainium2 Bass kernel for nn_FACoef.

Math: reference computes s_i(b) = sum of all entries of x_b^(i+2) for
i in 0..3, then out[b] = sum_ij coef[i,j] * s_i^(j+1) / n^(i+j+2).

Key identity: with the column-sum chain c_k = (x^T)^k 1 (per-sample
TensorE matvecs, x_b as the bf16 stationary operand), the needed sums
are s_i = 1^T c_{i+2}, i.e. the plain sum of entries of c_{i+2}. So the
whole kernel is 5 chain eras of weight-load-bound matvecs plus, per
tile, four tiny selector-stationary sum-matmuls that land s_0..s_3 on
psum partitions 0..3. No VectorE row-sum reduce (34us/core at 1x
tensor_reduce rate) and no per-sample dot era.

Input is cast to bf16 on the host and packed [N, BPC, N] (i, b, j) so
each x-tile DMA is 128 partitions x 8KB contiguous lines at ~full HBM
rate, and HBM traffic is halved vs fp32.

Sharding: pure data parallel - batch dim of x split across 8 cores,
256 samples per core. The tiny coef enters only through a host-scaled
[4,4] table W and per-row scale sc (n^beta balancing, so no fp32
overflow when raising s_i to the 4th power on device).
"""

import numpy as np

B, N = 2048, 128
ROWS, COLS = 4, 4
N_CORES = 8
BPC = B // N_CORES  # samples per core
S = 32              # samples per sbuf tile
T = BPC // S        # tiles per core

_cache = {}


def _patch_tail_drain():
    """walrus CoreV3 setupSyncWait rejects instructions carrying several
    semaphore waits; TileContext's kernel-tail drain collects one wait per
    unobserved logical proc. Split them one wait per drain instruction."""
    import concourse.tile as tile
    from concourse import mybir
    from concourse.vector_clock import ScopedClock

    if getattr(tile.TileContext, "_drain_split_patched", False):
        return

    def _drain_and_barrier(self, tick_clock, wait_clock):
        nc = self.nc
        drain_inst = nc.sync.drain()
        wait_clock.add_sem_waits(
            drain_inst.ins, ScopedClock({None: tick_clock.global_clock})
        )
        si = drain_inst.ins.sync_info
        waits = list(si.on_wait) if si is not None and si.on_wait else []
        if len(waits) > 1:
            drain_inst.ins.sync_info = mybir.SyncInfo(
                on_wait=[waits[0]], on_update=list(si.on_update or [])
            )
            for w in waits[1:]:
                extra = nc.sync.drain()
                extra.ins.sync_info = mybir.SyncInfo(on_wait=[w], on_update=[])

        nc.all_engine_barrier()
        assert self.sems is not None
        popped = nc._tile_sem_poison_stack.pop()
        assert popped is self._sem_poison
        nc.clear_and_free_semaphores(list(self.sems.allocated().values()))
        nc.all_engine_barrier()

    tile.TileContext._drain_and_barrier = _drain_and_barrier
    tile.TileContext._drain_split_patched = True


def _split_multi_waits(nc):
    """walrus accepts at most one sync wait per instruction (two for
    EventSemaphore). Hoist extra waits onto same-engine NOPs inserted
    immediately before the carrying instruction."""
    from concourse import mybir

    n_split = 0
    for bb in nc.main_func.blocks:
        new = []
        for inst in bb.instructions:
            si = inst.sync_info
            waits = list(si.on_wait) if si is not None and si.on_wait else []
            cap = 2 if isinstance(inst, mybir.InstEventSemaphore) else 1
            if len(waits) > cap:
                for k, w in enumerate(waits[:-cap]):
                    nop = mybir.InstNoOp(name=f"{inst.name}-wsplit{k}", ins=[], outs=[])
                    nop.engine = inst.engine
                    nop.sync_info = mybir.SyncInfo(on_wait=[w], on_update=[])
                    nc.register_instruction(nop)
                    new.append(nop)
                    n_split += 1
                inst.sync_info = mybir.SyncInfo(
                    on_wait=waits[-cap:], on_update=list(si.on_update or [])
                )
            new.append(inst)
        bb.instructions[:] = new
    return n_split


def _consolidate_pe_incs(nc):
    """Every TensorE matmul carries a +1 semaphore inc (~26 ns issue tail
    each). Consumers only wait at era boundaries, so batch the increments:
    strip per-mm incs and emit one accumulated inc at each waited value."""
    from concourse import mybir

    waited = {}
    for bb in nc.main_func.blocks:
        for ins in bb.instructions:
            si = ins.sync_info
            if si is None:
                continue
            for w in si.on_wait or []:
                waited.setdefault(w.ant_name, set()).add(w.wait_value)

    for bb in nc.main_func.blocks:
        pe_insts = [
            ins
            for ins in bb.instructions
            if isinstance(ins, mybir.InstMatmult)
            and ins.sync_info is not None
            and ins.sync_info.on_update
        ]
        by_sem = {}
        for ins in pe_insts:
            for u in ins.sync_info.on_update:
                if u.update_mode == "sem-inc":
                    by_sem.setdefault(u.ant_name, []).append((ins, u))
        import bisect

        for sem, pairs in by_sem.items():
            wvals = waited.get(sem, set())
            cum = 0
            kept_cums = []
            for idx, (ins, u) in enumerate(pairs):
                cum += u.update_value
                keep = cum in wvals or idx == len(pairs) - 1
                si = ins.sync_info
                others = [
                    x
                    for x in si.on_update
                    if not (x.ant_name == sem and x.update_mode == "sem-inc")
                ]
                if keep:
                    others.append(u)  # original +1 inc (hw requires value 1)
                    kept_cums.append(cum)
                ins.sync_info = mybir.SyncInfo(
                    on_wait=list(si.on_wait or []), on_update=others
                )
            # remap every wait on this sem from raw counts to kept-inc counts
            for bb2 in nc.main_func.blocks:
                for ins in bb2.instructions:
                    si = ins.sync_info
                    if si is None or not si.on_wait:
                        continue
                    changed = False
                    new_waits = []
                    for w in si.on_wait:
                        if w.ant_name == sem and w.wait_mode == "sem-ge-imm":
                            n = bisect.bisect_left(kept_cums, w.wait_value) + 1
                            assert n <= len(kept_cums), (sem, w.wait_value)
                            new_waits.append(
                                mybir.SyncWait(
                                    sync_type="semaphore",
                                    id=w.id,
                                    ant_name=sem,
                                    wait_mode="sem-ge-imm",
                                    wait_value=n,
                                )
                            )
                            changed = True
                        else:
                            new_waits.append(w)
                    if changed:
                        ins.sync_info = mybir.SyncInfo(
                            on_wait=new_waits, on_update=list(si.on_update or [])
                        )


def _build_nc(reps=1):
    import concourse.bass as bass
    import concourse.tile as tile
    from concourse import mybir

    _patch_tail_drain()
    f32 = mybir.dt.float32
    bf16 = mybir.dt.bfloat16
    OP = mybir.AluOpType

    nc = bass.Bass()
    x_in = nc.declare_dram_parameter("x", [N, BPC, N], bf16, isOutput=False)
    w_in = nc.declare_dram_parameter("w", [ROWS, COLS], f32, isOutput=False)
    sc_in = nc.declare_dram_parameter("sc", [ROWS, 1], f32, isOutput=False)
    y_out = nc.declare_dram_parameter("y", [1, BPC], f32, isOutput=True)

    with tile.TileContext(nc) as tc:
        with (
            tc.tile_pool(name="xp", bufs=4) as xp,
            tc.tile_pool(name="cbp", bufs=4) as cbp,
            tc.tile_pool(name="cfp", bufs=4) as cfp,
            tc.tile_pool(name="constp", bufs=1) as constp,
            tc.tile_pool(name="smallp", bufs=1) as smallp,
            tc.tile_pool(name="psp", bufs=4, space="PSUM") as psp,
            tc.tile_pool(name="pssp", bufs=2, space="PSUM") as pssp,
            tc.tile_pool(name="psfp", bufs=1, space="PSUM") as psfp,
        ):
            onesb = constp.tile([N, 1], bf16)
            nc.vector.memset(onesb[:], 1.0)
            ones4 = constp.tile([ROWS, 1], f32)
            nc.vector.memset(ones4[:], 1.0)
            # 4 selector stationaries: selt[:, k*ROWS+m] = 1.0 iff m==k.
            # Era-k sum-matmul uses selt[:, k*ROWS:(k+1)*ROWS] so the sum
            # of c_{k+2} lands on psum partition k (other rows += 0).
            selt = constp.tile([N, ROWS * ROWS], f32)
            nc.vector.memset(selt[:], 0.0)
            for k in range(ROWS):
                nc.vector.memset(selt[:, k * ROWS + k : k * ROWS + k + 1], 1.0)
            wt = constp.tile([ROWS, COLS], f32)
            nc.sync.dma_start(wt[:], w_in[:])
            sct = constp.tile([ROWS, 1], f32)
            nc.sync.dma_start(sct[:], sc_in[:])

            xf = {}

            def dma_tile(t):
                xf[t] = xp.tile([N, S, N], bf16, name="xt", tag="xt")
                nc.sync.dma_start(xf[t][:, :, :], x_in[:, t * S : (t + 1) * S, :])

            for _rep in range(reps):
                for t in (0, 1):
                    dma_tile(t)
                sg = smallp.tile([ROWS, BPC], f32, name="sg", tag="sg")

                # tiles processed in pairs with eras interleaved (a,b,a,b..):
                # tile b's era-k matmuls hide tile a's psum->sbuf copy
                # latency, so TensorE never stalls on an era boundary.
                for tp in range(T // 2):
                    pair = (2 * tp, 2 * tp + 1)
                    if tp + 1 < T // 2:
                        for t in (2 * tp + 2, 2 * tp + 3):
                            dma_tile(t)
                    cts, ctbs = {}, {}
                    for t in pair:
                        # ct: c_2..c_5 fp32 (sum-matmul moving operands)
                        cts[t] = cfp.tile([N, 4 * S], f32, name="ct", tag="ct")
                        # ctb: c_1..c_4 bf16 (next era's moving operands)
                        ctbs[t] = cbp.tile([N, 4 * S], bf16, name="ctb", tag="ctb")
                    # era 1: c_1 = x^T 1
                    for t in pair:
                        xt = xf[t]
                        ps = psp.tile([N, S], f32, name="ps", tag="ps")
                        for b in range(S):
                            nc.tensor.matmul(
                                ps[:, b : b + 1], xt[:, b, :], onesb[:, 0:1]
                            )
                        nc.scalar.copy(ctbs[t][:, 0:S], ps[:])
                    # eras 2-5: c_k = x^T c_{k-1}
                    for k in range(2, 6):
                        for t in pair:
                            xt = xf[t]
                            ps = psp.tile([N, S], f32, name="ps", tag="ps")
                            cb = ctbs[t]
                            for b in range(S):
                                nc.tensor.matmul(
                                    ps[:, b : b + 1],
                                    xt[:, b, :],
                                    cb[:, (k - 2) * S + b : (k - 2) * S + b + 1],
                                )
                            if k < 5:
                                nc.scalar.copy(
                                    ctbs[t][:, (k - 1) * S : k * S], ps[:]
                                )
                            nc.vector.tensor_copy(
                                cts[t][:, (k - 2) * S : (k - 1) * S], ps[:]
                            )
                    # sums: s_i = 1^T c_{i+2}, landed on psum partition i via
                    # the selector stationaries, accumulated in one psum tile
                    for t in pair:
                        ct = cts[t]
                        pss = pssp.tile([ROWS, S], f32, name="pss", tag="pss")
                        for k in range(ROWS):
                            nc.tensor.matmul(
                                pss[:, :],
                                selt[:, k * ROWS : (k + 1) * ROWS],
                                ct[:, k * S : (k + 1) * S],
                                start=(k == 0),
                                stop=(k == ROWS - 1),
                            )
                        nc.vector.tensor_copy(sg[:, t * S : (t + 1) * S], pss[:])

                # poly epilogue on [4, BPC]: out = sum_ij W[i,j] sig_i^(j+1)
                sig = smallp.tile([ROWS, BPC], f32, name="sig", tag="sig")
                nc.vector.tensor_scalar_mul(sig[:], sg[:], sct[:, 0:1])
                sig2 = smallp.tile([ROWS, BPC], f32, name="sig2", tag="sig2")
                nc.vector.tensor_mul(sig2[:], sig[:], sig[:])
                sig3 = smallp.tile([ROWS, BPC], f32, name="sig3", tag="sig3")
                nc.vector.tensor_mul(sig3[:], sig2[:], sig[:])
                sig4 = smallp.tile([ROWS, BPC], f32, name="sig4", tag="sig4")
                nc.vector.tensor_mul(sig4[:], sig2[:], sig2[:])
                acc1 = smallp.tile([ROWS, BPC], f32, name="acc1", tag="acc1")
                nc.vector.tensor_scalar_mul(acc1[:], sig[:], wt[:, 0:1])
                acc2 = smallp.tile([ROWS, BPC], f32, name="acc2", tag="acc2")
                nc.vector.scalar_tensor_tensor(
                    acc2[:], sig2[:], wt[:, 1:2], acc1[:], op0=OP.mult, op1=OP.add
                )
                acc3 = smallp.tile([ROWS, BPC], f32, name="acc3", tag="acc3")
                nc.vector.scalar_tensor_tensor(
                    acc3[:], sig3[:], wt[:, 2:3], acc2[:], op0=OP.mult, op1=OP.add
                )
                acc4 = smallp.tile([ROWS, BPC], f32, name="acc4", tag="acc4")
                nc.vector.scalar_tensor_tensor(
                    acc4[:], sig4[:], wt[:, 3:4], acc3[:], op0=OP.mult, op1=OP.add
                )
                psf = psfp.tile([1, BPC], f32, name="psf", tag="psf")
                nc.tensor.matmul(psf[0:1, :], ones4[:, 0:1], acc4[:])
                outt = smallp.tile([1, BPC], f32, name="outt", tag="outt")
                nc.vector.tensor_copy(outt[:], psf[:])
                nc.sync.dma_start(y_out[:], outt[:])

    _consolidate_pe_incs(nc)
    _split_multi_waits(nc)
    return nc


def _host_tables(coef):
    n = np.float64(N * N)
    ii = np.arange(ROWS, dtype=np.float64)[:, None]
    jj = np.arange(COLS, dtype=np.float64)[None, :]
    beta = (ii + 3.0) / 4.0
    w = (coef.astype(np.float64) * n ** (beta * (jj + 1.0) - (ii + jj + 2.0))).astype(
        np.float32
    )
    sc = (n ** (-beta[:, 0:1])).astype(np.float32)
    return w, sc


def _in_maps(x, coef):
    import ml_dtypes

    w, sc = _host_tables(np.asarray(coef))
    x = np.asarray(x, dtype=np.float32)
    maps = []
    for c in range(N_CORES):
        xc = x[c * BPC : (c + 1) * BPC].astype(ml_dtypes.bfloat16)
        xp = np.ascontiguousarray(xc.transpose(1, 0, 2))  # [N, BPC, N] (i,b,j)
        maps.append({"x": xp, "w": w, "sc": sc})
    return maps


def kernel(x, coef):
    from concourse.bass_utils import run_bass_kernel_spmd

    if "nc" not in _cache:
        _cache["nc"] = _build_nc()
    nc = _cache["nc"]

    in_maps = _in_maps(x, coef)
    res = run_bass_kernel_spmd(nc, in_maps, list(range(N_CORES)))
    y = np.concatenate(
        [np.asarray(res.results[c]["y"]).reshape(-1) for c in range(N_CORES)]
    )
    return y.astype(np.float32)
